# revision 1
# baseline (speedup 1.0000x reference)
"""Trainium2 Bass kernel for nn_DynamicHead (varying-coefficient spline MLP).

Math: basis(t) = [1,t,t^2,t^3, relu(t-k_j)^3 ...] (12 fns, 8 knots at j/9).
Each vc_layer: out = sum_s basis_s * (x @ W_s) + basis @ b.
Within knot segment m (t in [k_m, k_{m+1})), relu terms are plain cubics, so
basis collapses to powers [1,t,t^2,t^3] against segment-combined weights
C[m,p] = sum_s gamma[m,p,s] W_s.  Each layer is then a K=4*256 matmul over
z[(p,i), b] = t^p * x[i, b] for samples grouped by segment.

Host: sort samples by segment, deal round-robin across 8 cores (uniform
segment sizes), pad each per-core segment to CAP; prepack weights into the
exact SBUF tile layouts. Device (per core, SPMD): for each segment, 3 layers
of fp32r matmuls (feature-major activations), DVE builds z tiles from
DMA-broadcast power rows, ScalarE does relu+PSUM evacuation; final layer
(out_dim=1) contracts features first, then basis via a 4-partition
elementwise + ones-matmul reduction. Segment-skewed software pipeline.
"""
import os
import sys
import types

for _p in ('/opt/trn_rl_repo', '/root/.axon_site/_ro/trn_rl_repo'):
    if _p not in sys.path:
        sys.path.append(_p)

import numpy as np
import concourse.bass as bass
import concourse.tile as tile
from concourse import bacc, mybir
from concourse import bass_utils

F32 = mybir.dt.float32
F32R = mybir.dt.float32r
RELU = mybir.ActivationFunctionType.Relu
COPY = mybir.ActivationFunctionType.Copy
IDENT = mybir.ActivationFunctionType.Identity

B, D, NSEG, NSB = 32768, 256, 9, 4
N_CORES = 8
KNOTS = np.array([i / 9.0 for i in range(1, 9)], dtype=np.float64)
SDIM = 12
NKT = NSB * D // 128                   # 8 k-tiles of 128

# set True by test harness for a profiled run
TRACE = False
LAST_EXEC_NS = None
LAST_MEAN_EXEC_NS = None
LAST_RES = None

_PROG_CACHE = {}

if os.environ.get("BASS_LDW_OPT") == "1":
    _orig_run_command = bass_utils.run_command

    def _run_command_ldw(argv, **kw):
        argv = ["--enable-ldw-opt=true" if a == "--enable-ldw-opt=false" else a
                for a in argv]
        return _orig_run_command(argv, **kw)

    bass_utils.run_command = _run_command_ldw


def _register_ntff_hook():
    try:
        import antenv.axon_hooks  # noqa: F401
        return
    except ImportError:
        pass
    try:
        from trn_agent_boot.trn_boot import _ntff_profile_via_ctypes
        hook = _ntff_profile_via_ctypes('/opt/axon/libaxon_pjrt.so')
        mod = types.ModuleType('antenv.axon_hooks')
        mod.get_axon_ntff_profile_hook = lambda: hook
        sys.modules['antenv.axon_hooks'] = mod
    except Exception:
        pass


def _gamma() -> np.ndarray:
    """(NSEG, NSB, SDIM): basis -> per-segment cubic coefficients."""
    g = np.zeros((NSEG, NSB, SDIM), dtype=np.float64)
    for m in range(NSEG):
        for p in range(NSB):
            g[m, p, p] = 1.0
        for j in range(1, 9):          # spline s = 3 + j, knot k = j/9
            if j <= m:
                k = KNOTS[j - 1]
                g[m, 0, 3 + j] = -k ** 3
                g[m, 1, 3 + j] = 3 * k ** 2
                g[m, 2, 3 + j] = -3 * k
                g[m, 3, 3 + j] = 1.0
    return g


def _chunks(cap):
    out, off = [], 0
    while off < cap:
        n = min(512, cap - off)
        out.append((off, n))
        off += n
    return out


def _build_program(cap):
    """Build + compile the SPMD single-core program for per-segment capacity cap."""
    bp = cap * NSEG                     # padded per-core batch
    nc = bacc.Bacc("TRN2", target_bir_lowering=False, debug=False,
                   num_devices=N_CORES)

    # cw: per (layer, seg) prepacked (128, 2304): 8 k-tile blocks of 256 (o)
    # cols + bias block (partitions 0..3) at cols 2048..2303.
    xT_ap = nc.dram_tensor("xT", [D, bp], F32, kind="ExternalInput").ap()
    tp_ap = nc.dram_tensor("tp", [NSB, bp], F32, kind="ExternalInput").ap()
    c0w_ap = nc.dram_tensor("c0w", [NSEG, 128, (NKT + 1) * D], F32, kind="ExternalInput").ap()
    c1w_ap = nc.dram_tensor("c1w", [NSEG, 128, (NKT + 1) * D], F32, kind="ExternalInput").ap()
    # c2: prepacked (128, 9): cols h*4..h*4+3 = c2w k-tile h, col 8 (parts 0..3) = bias
    c2_ap = nc.dram_tensor("c2", [NSEG, 128, 2 * NSB + 1], F32, kind="ExternalInput").ap()
    ones_ap = nc.dram_tensor("ones4", [NSB, 1], F32, kind="ExternalInput").ap()
    out_ap = nc.dram_tensor("out", [1, bp], F32, kind="ExternalOutput").ap()

    cw_ap = (c0w_ap, c1w_ap)

    with tile.TileContext(nc) as tc:
        with (
            tc.tile_pool(name="act", bufs=1) as actp,
            tc.tile_pool(name="bc", bufs=1) as bcp,
            tc.tile_pool(name="z", bufs=1) as zp,
            tc.tile_pool(name="w", bufs=1) as wp,
            tc.tile_pool(name="sm", bufs=1) as smp,
            tc.tile_pool(name="pm", bufs=1, space="PSUM") as pmp,
            tc.tile_pool(name="pq", bufs=1, space="PSUM") as pqp,
        ):
            ones4 = smp.tile([NSB, 1], F32R, name="ones4", tag="ones4")
            nc.gpsimd.dma_start(ones4[:, :], ones_ap[:, :].bitcast(F32R))

            xin, x1, x2, bcast, tps = {}, {}, {}, {}, {}

            def load_seg(s):
                sl = slice(s * cap, (s + 1) * cap)
                # both feature halves in one tile: cols h*cap..(h+1)*cap
                xt = actp.tile([128, 2 * cap], F32R, name=f"xin{s}",
                               tag="xin", bufs=3)
                nc.gpsimd.dma_start(
                    xt[:, :].rearrange("p (h b) -> p h b", h=2),
                    xT_ap[:, sl].rearrange("(h p) b -> p h b", p=128).bitcast(F32R))
                xin[s] = xt
                # power rows broadcast to 128 partitions, h-doubled:
                # block (p-1): [bc_p | bc_p] each of width cap
                bt = bcp.tile([128, (NSB - 1) * 2 * cap], F32R,
                              name=f"bc{s}", tag="bc", bufs=3)
                bt4 = bt[:, :].rearrange("q (p h b) -> q p h b", p=NSB - 1, h=2)
                for h in range(2):
                    nc.gpsimd.dma_start(
                        bt4[:, :, h, :],
                        tp_ap[1:NSB, sl].partition_broadcast(128).bitcast(F32R))
                bcast[s] = bt
                tps[s] = smp.tile([NSB, cap], F32R, name=f"tp{s}", tag="tp", bufs=3)
                nc.gpsimd.dma_start(tps[s][:, :], tp_ap[:, sl].bitcast(F32R))

            def vc_layer(s, L, xin_t, store):
                """layers 0/1: (o,b) = relu(C.T @ z + Cb.T @ tp), feature-major"""
                wt = wp.tile([128, (NKT + 1) * D], F32R, name=f"w{L}_{s}",
                             tag="w", bufs=4)
                nc.sync.dma_start(wt[:, :], cw_ap[L][s].bitcast(F32R))

                zt = zp.tile([128, (NSB - 1) * 2 * cap], F32R,
                             name=f"z{L}_{s}", tag="z", bufs=3)
                for p in range(1, NSB):
                    blk = (p - 1) * 2 * cap
                    nc.vector.tensor_mul(
                        zt[:, blk:blk + 2 * cap],
                        xin_t[:, :],
                        bcast[s][:, blk:blk + 2 * cap])

                outs = []
                for m in range(2):
                    for off, n in _chunks(cap):
                        ps = pmp.tile([128, n], F32, name=f"pm{L}_{s}_{m}_{off}",
                                      tag="pm", bufs=6)
                        for kt in range(NKT):
                            p, h = divmod(kt, 2)
                            if p == 0:
                                rhs = xin_t[:, h * cap + off:h * cap + off + n]
                            else:
                                blk = ((p - 1) * 2 + h) * cap
                                rhs = zt[:, blk + off:blk + off + n]
                            nc.tensor.matmul(
                                ps[:, :],
                                wt[:, kt * D + m * 128:kt * D + (m + 1) * 128],
                                rhs, start=(kt == 0), stop=False)
                        nc.tensor.matmul(ps[:, :],
                                         wt[0:NSB, NKT * D + m * 128:NKT * D + (m + 1) * 128],
                                         tps[s][:, off:off + n],
                                         start=False, stop=True)
                        outs.append((m, off, n, ps))
                xo = actp.tile([128, 2 * cap], F32R, name=f"x{L + 1}_{s}",
                               tag=f"xo{L}", bufs=3)
                for m, off, n, ps in outs:
                    nc.scalar.activation(xo[:, m * cap + off:m * cap + off + n],
                                         ps[:, :], RELU)
                store[s] = xo

            def head_layer(s):
                """layer 2 (out_dim=1): q=C2.T@x2 (+b2), out = ones.T @ (q*tp)"""
                c2t = smp.tile([128, 2 * NSB + 1], F32R, name=f"c2_{s}",
                               tag="c2", bufs=3)
                nc.gpsimd.dma_start(c2t[:, :], c2_ap[s].bitcast(F32R))
                for off, n in _chunks(cap):
                    psq = pqp.tile([NSB, n], F32, name=f"pq{s}_{off}", tag="pq", bufs=1)
                    for h in range(2):
                        nc.tensor.matmul(psq[:, :],
                                         c2t[:, h * NSB:(h + 1) * NSB],
                                         x2[s][:, h * cap + off:h * cap + off + n],
                                         start=(h == 0), stop=(h == 1))
                    qb = smp.tile([NSB, n], F32R, name=f"qb{s}_{off}", tag="qb", bufs=3)
                    nc.scalar.activation(qb[:, :], psq[:, :], IDENT,
                                         bias=c2t[0:NSB, 2 * NSB:2 * NSB + 1].bitcast(F32))
                    rq = smp.tile([NSB, n], F32R, name=f"rq{s}_{off}", tag="rq", bufs=3)
                    nc.vector.tensor_mul(rq[:, :], qb[:, :],
                                         tps[s][:, off:off + n])
                    psr = pqp.tile([1, n], F32, name=f"pr{s}_{off}", tag="pr", bufs=1)
                    nc.tensor.matmul(psr[:, :], ones4[:, :], rq[:, :],
                                     start=True, stop=True)
                    orow = smp.tile([1, n], F32, name=f"or{s}_{off}", tag="or", bufs=3)
                    nc.scalar.activation(orow[:, :], psr[:, :], COPY)
                    nc.gpsimd.dma_start(out_ap[0:1, s * cap + off:s * cap + off + n],
                                        orow[:, :])

            # segment-skewed software pipeline: L0(s+1) overlaps L1(s)/L2(s-1)
            for step in range(NSEG + 2):
                if step < NSEG:
                    load_seg(step)
                    vc_layer(step, 0, xin[step], x1)
                    xin.pop(step)
                if 1 <= step < NSEG + 1:
                    vc_layer(step - 1, 1, x1[step - 1], x2)
                    x1.pop(step - 1)
                if step >= 2:
                    head_layer(step - 2)
                    x2.pop(step - 2)

    nc.compile()
    return nc


def _prep_host(treatment, features, W0, b0, W1, b1, W2, b2):
    t = np.asarray(treatment, dtype=np.float32)
    x = np.asarray(features, dtype=np.float32)
    seg = np.searchsorted(KNOTS.astype(np.float32), t, side='right')

    # deal each segment round-robin across cores
    core_of = np.empty(B, dtype=np.int64)
    pos_of = np.empty(B, dtype=np.int64)
    counts = np.zeros((N_CORES, NSEG), dtype=np.int64)
    for m in range(NSEG):
        idx = np.nonzero(seg == m)[0]
        for c in range(N_CORES):
            sub = idx[c::N_CORES]
            core_of[sub] = c
            pos_of[sub] = np.arange(len(sub))
            counts[c, m] = len(sub)
    maxn = int(counts.max())
    cap = max(512, ((maxn + 127) // 128) * 128)
    bp = cap * NSEG

    gather = np.full((N_CORES, bp), -1, dtype=np.int64)
    slot = seg * cap + pos_of
    gather[core_of, slot] = np.arange(B)

    xT = np.zeros((N_CORES, D, bp), dtype=np.float32)
    tp = np.zeros((N_CORES, NSB, bp), dtype=np.float32)
    for c in range(N_CORES):
        v = gather[c] >= 0
        gi = gather[c][v]
        xT[c][:, v] = x[gi].T
        tv = t[gi].astype(np.float64)
        tp[c][:, v] = np.stack([tv ** p for p in range(NSB)]).astype(np.float32)

    g = _gamma()
    cw = []
    for W, b in ((W0, b0), (W1, b1)):
        Ws = np.asarray(W, dtype=np.float64).reshape(SDIM, D, D)
        c = np.einsum('mps,sio->mpio', g, Ws).reshape(NSEG, NSB * D, D)
        cb = np.einsum('mps,so->mpo', g, np.asarray(b, np.float64))
        packed = np.zeros((NSEG, 128, (NKT + 1) * D), dtype=np.float32)
        for kt in range(NKT):
            packed[:, :, kt * D:(kt + 1) * D] = c[:, kt * 128:(kt + 1) * 128, :]
        packed[:, 0:NSB, NKT * D:] = cb
        cw.append(packed)
    c2w = np.einsum('mps,si->mip', g, np.asarray(W2, np.float64))   # (9, 256, 4)
    c2b = np.einsum('mps,s->mp', g, np.asarray(b2, np.float64)[:, 0])
    c2 = np.zeros((NSEG, 128, 2 * NSB + 1), dtype=np.float32)
    for h in range(2):
        c2[:, :, h * NSB:(h + 1) * NSB] = c2w[:, h * 128:(h + 1) * 128, :]
    c2[:, 0:NSB, 2 * NSB] = c2b

    shared = dict(c0w=np.ascontiguousarray(cw[0]), c1w=np.ascontiguousarray(cw[1]),
                  c2=np.ascontiguousarray(c2),
                  ones4=np.ones((NSB, 1), np.float32))
    in_maps = [dict(shared, xT=np.ascontiguousarray(xT[c]),
                    tp=np.ascontiguousarray(tp[c])) for c in range(N_CORES)]
    return cap, in_maps, gather


def kernel(treatment, features, W0, b0, W1, b1, W2, b2):
    global LAST_EXEC_NS, LAST_MEAN_EXEC_NS, LAST_RES
    cap, in_maps, gather = _prep_host(treatment, features, W0, b0, W1, b1, W2, b2)

    if cap not in _PROG_CACHE:
        _PROG_CACHE[cap] = _build_program(cap)
    nc = _PROG_CACHE[cap]

    if TRACE:
        _register_ntff_hook()
    res = bass_utils.run_bass_kernel_spmd(
        nc, in_maps, core_ids=list(range(N_CORES)), trace=TRACE)
    LAST_EXEC_NS = res.exec_time_ns
    LAST_MEAN_EXEC_NS = res.mean_exec_time_ns
    LAST_RES = res

    out = np.empty((B,), dtype=np.float32)
    for c in range(N_CORES):
        row = res.results[c]["out"][0]
        v = gather[c] >= 0
        out[gather[c][v]] = row[v]
    return out.reshape(B, 1)



# revision 2
# speedup vs baseline: 1.7975x; 1.7975x over previous
"""Trainium2 Bass kernel for nn_DynamicHead (varying-coefficient spline MLP).

Math: basis(t) = [1,t,t^2,t^3, relu(t-k_j)^3 ...] (12 fns, 8 knots at j/9).
Each vc_layer: out = sum_s basis_s * (x @ W_s) + basis @ b.
Within knot segment m, the function is an exact cubic in dt = t - t0_m
(segment center).  The cubic term's relative contribution is O(h^3) with
h = 1/18, so we fold it into the linear term via the Chebyshev minimax
identity dt^3 ~ (3h^2/4) dt (error h^3/4 ~ 4e-5 relative) and keep a
QUADRATIC basis [1, dt, dt^2]: K = 3*256 = 768 per layer instead of 1024.
All matmul/DVE operands are bf16 (PSUM accumulates fp32): rel-err ~1e-2
vs the 2e-2 gate, 1 cycle/row matmuls, 2x DVE, half DMA.

Host: sort samples by segment, deal round-robin across 8 cores (uniform
per-core segment sizes +-1), pad each segment to a multiple of 8 (ragged
caps); prepack centered-quadratic combined weights into exact SBUF tile
layouts. Device (per core, SPMD): one-time loads (dt-row broadcast T1,
power rows tps, bias/head tables), then per segment: 3 layers of bf16
matmuls (feature-major), DVE builds z1 = x*dt, z2 = z1*dt chained,
ScalarE does relu+PSUM evacuation with the K=3 bias matmul folded into
the accumulation group; head (out_dim=1) contracts features to [3,cap],
adds bias via per-partition ACT bias, multiplies by tps and reduces with
a ones-matmul. Segment-skewed software pipeline keeps PE continuously
busy (HAM stays at full clock).
"""
import os
import sys
import types

for _p in ('/opt/trn_rl_repo', '/root/.axon_site/_ro/trn_rl_repo'):
    if _p not in sys.path:
        sys.path.append(_p)

import numpy as np
import ml_dtypes
import concourse.bass as bass
import concourse.tile as tile
from concourse import bacc, mybir
from concourse import bass_utils

F32 = mybir.dt.float32
BF16 = mybir.dt.bfloat16
NPBF = ml_dtypes.bfloat16
RELU = mybir.ActivationFunctionType.Relu
COPY = mybir.ActivationFunctionType.Copy
IDENT = mybir.ActivationFunctionType.Identity

B, D, NSEG = 32768, 256, 9
NP = 3                                  # quadratic centered basis [1, dt, dt^2]
KT = NP * D // 128                      # 6 k-tiles of 128
N_CORES = 8
KNOTS = np.array([i / 9.0 for i in range(1, 9)], dtype=np.float64)
SDIM = 12
T0 = np.array([(m + 0.5) / 9.0 for m in range(NSEG)])   # segment centers
H = 0.5 / 9.0                                           # segment half-width

TRACE = False
LAST_EXEC_NS = None
LAST_MEAN_EXEC_NS = None
LAST_RES = None

_PROG_CACHE = {}

if os.environ.get("BASS_LDW_OPT") == "1":
    _orig_run_command = bass_utils.run_command

    def _run_command_ldw(argv, **kw):
        argv = ["--enable-ldw-opt=true" if a == "--enable-ldw-opt=false" else a
                for a in argv]
        return _orig_run_command(argv, **kw)

    bass_utils.run_command = _run_command_ldw


def _register_ntff_hook():
    try:
        import antenv.axon_hooks  # noqa: F401
        return
    except ImportError:
        pass
    try:
        from trn_agent_boot.trn_boot import _ntff_profile_via_ctypes
        hook = _ntff_profile_via_ctypes('/opt/axon/libaxon_pjrt.so')
        mod = types.ModuleType('antenv.axon_hooks')
        mod.get_axon_ntff_profile_hook = lambda: hook
        sys.modules['antenv.axon_hooks'] = mod
    except Exception:
        pass


def _gamma4() -> np.ndarray:
    """(NSEG, 4, SDIM): basis -> per-segment cubic coefficients (t-basis)."""
    g = np.zeros((NSEG, 4, SDIM), dtype=np.float64)
    for m in range(NSEG):
        for p in range(4):
            g[m, p, p] = 1.0
        for j in range(1, 9):          # spline s = 3 + j, knot k = j/9
            if j <= m:
                k = KNOTS[j - 1]
                g[m, 0, 3 + j] = -k ** 3
                g[m, 1, 3 + j] = 3 * k ** 2
                g[m, 2, 3 + j] = -3 * k
                g[m, 3, 3 + j] = 1.0
    return g


def _recenter(c4, t0):
    """cubic coeffs (4, ...) in t-basis -> quadratic (3, ...) in dt-basis.

    Taylor recenter at t0, then Chebyshev-fold the exact cubic term:
    dt^3 = (3h^2/4) dt + (h^3/4) T3(dt/h); drop the T3 remainder."""
    from math import comb
    c = np.zeros((4,) + c4.shape[1:])
    for q in range(4):
        for p in range(q, 4):
            c[q] += comb(p, q) * (t0 ** (p - q)) * c4[p]
    out = c[:3].copy()
    out[1] += 0.75 * H * H * c[3]
    return out


def _build_program(caps):
    """Build + compile the SPMD single-core program for per-seg caps tuple."""
    caps = tuple(int(c) for c in caps)
    offs = [0]
    for c in caps:
        offs.append(offs[-1] + c)
    bp = offs[-1]
    nc = bacc.Bacc("TRN2", target_bir_lowering=False, debug=False,
                   num_devices=N_CORES)

    xT_ap = nc.dram_tensor("xT", [D, bp], BF16, kind="ExternalInput").ap()
    tp_ap = nc.dram_tensor("tp", [NP, bp], BF16, kind="ExternalInput").ap()
    # cw: per (layer, seg) prepacked (128, KT*256): k-tile kt at cols
    # kt*256..(kt+1)*256; within: half mo at cols mo*128.
    c0w_ap = nc.dram_tensor("c0w", [NSEG, 128, KT * D], BF16, kind="ExternalInput").ap()
    c1w_ap = nc.dram_tensor("c1w", [NSEG, 128, KT * D], BF16, kind="ExternalInput").ap()
    # bias lhsT for both layers: [3, (L*NSEG + s)*256 + mo*128 ...]
    cbw_ap = nc.dram_tensor("cbw", [NP, 2 * NSEG * D], BF16, kind="ExternalInput").ap()
    # head lhsT: [128, s*6 + h*3 + p]
    c2_ap = nc.dram_tensor("c2", [128, NSEG * 2 * NP], BF16, kind="ExternalInput").ap()
    c2b_ap = nc.dram_tensor("c2b", [NP, NSEG], F32, kind="ExternalInput").ap()
    ones_ap = nc.dram_tensor("ones3", [NP, 1], BF16, kind="ExternalInput").ap()
    out_ap = nc.dram_tensor("out", [1, bp], F32, kind="ExternalOutput").ap()

    cw_ap = (c0w_ap, c1w_ap)

    with tile.TileContext(nc) as tc:
        with (
            tc.tile_pool(name="act", bufs=1) as actp,
            tc.tile_pool(name="bc", bufs=1) as bcp,
            tc.tile_pool(name="z", bufs=1) as zp,
            tc.tile_pool(name="w", bufs=1) as wp,
            tc.tile_pool(name="sm", bufs=1) as smp,
            tc.tile_pool(name="pm", bufs=1, space="PSUM") as pmp,
            tc.tile_pool(name="pq", bufs=1, space="PSUM") as pqp,
        ):
            # ---- one-time loads ----
            # dt row broadcast to 128 partitions; seg 0 first so z(0) can start
            t1 = bcp.tile([128, bp], BF16, name="t1", tag="t1")
            nc.gpsimd.dma_start(t1[:, 0:caps[0]],
                                tp_ap[1:2, 0:caps[0]].partition_broadcast(128))
            nc.gpsimd.dma_start(t1[:, caps[0]:bp],
                                tp_ap[1:2, caps[0]:bp].partition_broadcast(128))
            tps = smp.tile([NP, bp], BF16, name="tps", tag="tps")
            nc.sync.dma_start(tps[:, :], tp_ap[:, :])
            cbw = smp.tile([NP, 2 * NSEG * D], BF16, name="cbw", tag="cbw")
            nc.sync.dma_start(cbw[:, :], cbw_ap[:, :])
            c2t = smp.tile([128, NSEG * 2 * NP], BF16, name="c2t", tag="c2t")
            nc.sync.dma_start(c2t[:, :], c2_ap[:, :])
            c2b = smp.tile([NP, NSEG], F32, name="c2b", tag="c2b")
            nc.sync.dma_start(c2b[:, :], c2b_ap[:, :])
            ones3 = smp.tile([NP, 1], BF16, name="ones3", tag="ones3")
            nc.sync.dma_start(ones3[:, :], ones_ap[:, :])
            out_all = smp.tile([1, bp], F32, name="out_all", tag="out_all")

            xin, x1, x2 = {}, {}, {}

            def load_seg(s):
                cap, off = caps[s], offs[s]
                # both feature halves in one tile: cols h*cap..(h+1)*cap
                xt = actp.tile([128, 2 * cap], BF16, name=f"xin{s}",
                               tag="xin", bufs=3)
                nc.scalar.dma_start(
                    xt[:, :].rearrange("p (h b) -> p h b", h=2),
                    xT_ap[:, off:off + cap].rearrange("(h p) b -> p h b", p=128))
                xin[s] = xt

            def vc_layer(s, L, xin_t, store):
                """layers 0/1: (o,b) = relu(C.T @ [x;z1;z2] + Cb.T @ tps)"""
                cap, off = caps[s], offs[s]
                wt = wp.tile([128, KT * D], BF16, name=f"w{L}_{s}",
                             tag="w", bufs=4)
                nc.sync.dma_start(wt[:, :], cw_ap[L][s])

                z1 = zp.tile([128, 2 * cap], BF16, name=f"z1_{L}_{s}",
                             tag="z1", bufs=2)
                z2 = zp.tile([128, 2 * cap], BF16, name=f"z2_{L}_{s}",
                             tag="z2", bufs=2)
                for h in range(2):
                    nc.vector.tensor_mul(z1[:, h * cap:(h + 1) * cap],
                                         xin_t[:, h * cap:(h + 1) * cap],
                                         t1[:, off:off + cap])
                for h in range(2):
                    nc.vector.tensor_mul(z2[:, h * cap:(h + 1) * cap],
                                         z1[:, h * cap:(h + 1) * cap],
                                         t1[:, off:off + cap])
                rhs_of = [xin_t, xin_t, z1, z1, z2, z2]
                xo = actp.tile([128, 2 * cap], BF16, name=f"x{L + 1}_{s}",
                               tag=f"xo{L}", bufs=3)
                for m in range(2):
                    ps = pmp.tile([128, cap], F32, name=f"pm{L}_{s}_{m}",
                                  tag="pm", bufs=6)
                    for kt in range(KT):
                        h = kt % 2
                        nc.tensor.matmul(
                            ps[:, :],
                            wt[:, kt * D + m * 128:kt * D + (m + 1) * 128],
                            rhs_of[kt][:, h * cap:(h + 1) * cap],
                            start=(kt == 0), stop=False)
                    nc.tensor.matmul(
                        ps[:, :],
                        cbw[0:NP, (L * NSEG + s) * D + m * 128:
                            (L * NSEG + s) * D + (m + 1) * 128],
                        tps[0:NP, off:off + cap],
                        start=False, stop=True)
                    nc.scalar.activation(xo[:, m * cap:(m + 1) * cap],
                                         ps[:, :], RELU)
                store[s] = xo

            def head_layer(s):
                """layer 2 (out_dim=1): q = C2.T @ x2 (+b2); out = ones.T @ (q*tps)"""
                cap, off = caps[s], offs[s]
                psq = pqp.tile([NP, cap], F32, name=f"pq{s}", tag="pq", bufs=1)
                for h in range(2):
                    nc.tensor.matmul(psq[:, :],
                                     c2t[:, s * 2 * NP + h * NP:
                                         s * 2 * NP + (h + 1) * NP],
                                     x2[s][:, h * cap:(h + 1) * cap],
                                     start=(h == 0), stop=(h == 1))
                qb = smp.tile([NP, cap], BF16, name=f"qb{s}", tag="qb", bufs=3)
                nc.scalar.activation(qb[:, :], psq[:, :], IDENT,
                                     bias=c2b[0:NP, s:s + 1])
                rq = smp.tile([NP, cap], BF16, name=f"rq{s}", tag="rq", bufs=3)
                nc.vector.tensor_mul(rq[:, :], qb[:, :],
                                     tps[0:NP, off:off + cap])
                psr = pqp.tile([1, cap], F32, name=f"pr{s}", tag="pr", bufs=1)
                nc.tensor.matmul(psr[:, :], ones3[:, :], rq[:, :],
                                 start=True, stop=True)
                nc.scalar.activation(out_all[0:1, off:off + cap],
                                     psr[:, :], COPY)

            # segment-skewed software pipeline: L0(s+1) overlaps L1(s)/L2(s-1)
            for step in range(NSEG + 2):
                if step < NSEG:
                    load_seg(step)
                    vc_layer(step, 0, xin[step], x1)
                    xin.pop(step)
                if 1 <= step < NSEG + 1:
                    vc_layer(step - 1, 1, x1[step - 1], x2)
                    x1.pop(step - 1)
                if step >= 2:
                    head_layer(step - 2)
                    x2.pop(step - 2)

            nc.sync.dma_start(out_ap[:, :], out_all[:, :])

    nc.compile()
    return nc


def _prep_host(treatment, features, W0, b0, W1, b1, W2, b2):
    t = np.asarray(treatment, dtype=np.float64)
    x = np.asarray(features, dtype=np.float32)
    seg = np.searchsorted(KNOTS.astype(np.float32), t.astype(np.float32),
                          side='right')

    # deal each segment round-robin across cores (per-core counts within 1)
    core_of = np.empty(B, dtype=np.int64)
    pos_of = np.empty(B, dtype=np.int64)
    counts = np.zeros((N_CORES, NSEG), dtype=np.int64)
    for m in range(NSEG):
        idx = np.nonzero(seg == m)[0]
        for c in range(N_CORES):
            sub = idx[c::N_CORES]
            core_of[sub] = c
            pos_of[sub] = np.arange(len(sub))
            counts[c, m] = len(sub)
    caps = tuple(int(max(8, -(-int(counts[:, m].max()) // 8) * 8))
                 for m in range(NSEG))
    assert max(caps) <= 512, caps
    offs = np.concatenate([[0], np.cumsum(caps)])
    bp = int(offs[-1])

    gather = np.full((N_CORES, bp), -1, dtype=np.int64)
    slot = offs[seg] + pos_of
    gather[core_of, slot] = np.arange(B)

    dt_full = t - T0[seg]
    xT = np.zeros((N_CORES, D, bp), dtype=NPBF)
    tp = np.zeros((N_CORES, NP, bp), dtype=NPBF)
    for c in range(N_CORES):
        v = gather[c] >= 0
        gi = gather[c][v]
        xT[c][:, v] = x[gi].T.astype(NPBF)
        dv = dt_full[gi]
        tp[c][:, v] = np.stack([np.ones_like(dv), dv, dv * dv]).astype(NPBF)

    g = _gamma4()
    cw, cbs = [], []
    for W, b in ((W0, b0), (W1, b1)):
        Ws = np.asarray(W, dtype=np.float64).reshape(SDIM, D, D)
        c4 = np.einsum('mps,sio->mpio', g, Ws)
        cb4 = np.einsum('mps,so->mpo', g, np.asarray(b, np.float64))
        c3 = np.stack([_recenter(c4[m], T0[m]) for m in range(NSEG)])
        cb3 = np.stack([_recenter(cb4[m], T0[m]) for m in range(NSEG)])
        c3r = c3.reshape(NSEG, NP * D, D)
        packed = np.zeros((NSEG, 128, KT * D), dtype=NPBF)
        for kt in range(KT):
            packed[:, :, kt * D:(kt + 1) * D] = \
                c3r[:, kt * 128:(kt + 1) * 128, :].astype(NPBF)
        cw.append(packed)
        cbs.append(cb3)                 # (NSEG, 3, 256)
    cbw = np.zeros((NP, 2 * NSEG * D), dtype=NPBF)
    for L in range(2):
        for m in range(NSEG):
            cbw[:, (L * NSEG + m) * D:(L * NSEG + m + 1) * D] = \
                cbs[L][m].astype(NPBF)

    c4h = np.einsum('mps,si->mpi', g, np.asarray(W2, np.float64))  # (9,4,256)
    c3h = np.stack([_recenter(c4h[m], T0[m]) for m in range(NSEG)])  # (9,3,256)
    cb4h = np.einsum('mps,s->mp', g, np.asarray(b2, np.float64)[:, 0])
    cb3h = np.stack([_recenter(cb4h[m][:, None], T0[m])[:, 0]
                     for m in range(NSEG)])                         # (9,3)
    c2 = np.zeros((128, NSEG * 2 * NP), dtype=NPBF)
    for m in range(NSEG):
        for h in range(2):
            c2[:, m * 2 * NP + h * NP:m * 2 * NP + (h + 1) * NP] = \
                c3h[m][:, h * 128:(h + 1) * 128].T.astype(NPBF)
    c2b = np.ascontiguousarray(cb3h.T.astype(np.float32))           # (3, 9)

    shared = dict(c0w=np.ascontiguousarray(cw[0]),
                  c1w=np.ascontiguousarray(cw[1]),
                  cbw=cbw, c2=c2, c2b=c2b,
                  ones3=np.ones((NP, 1), NPBF))
    in_maps = [dict(shared, xT=np.ascontiguousarray(xT[c]),
                    tp=np.ascontiguousarray(tp[c])) for c in range(N_CORES)]
    return caps, in_maps, gather


def kernel(treatment, features, W0, b0, W1, b1, W2, b2):
    global LAST_EXEC_NS, LAST_MEAN_EXEC_NS, LAST_RES
    caps, in_maps, gather = _prep_host(treatment, features, W0, b0, W1, b1,
                                       W2, b2)

    if caps not in _PROG_CACHE:
        _PROG_CACHE[caps] = _build_program(caps)
    nc = _PROG_CACHE[caps]

    if TRACE:
        _register_ntff_hook()
    res = bass_utils.run_bass_kernel_spmd(
        nc, in_maps, core_ids=list(range(N_CORES)), trace=TRACE)
    LAST_EXEC_NS = res.exec_time_ns
    LAST_MEAN_EXEC_NS = res.mean_exec_time_ns
    LAST_RES = res

    out = np.empty((B,), dtype=np.float32)
    for c in range(N_CORES):
        row = res.results[c]["out"][0]
        v = gather[c] >= 0
        out[gather[c][v]] = row[v]
    return out.reshape(B, 1)


# revision 11
# speedup vs baseline: 2.1331x; 1.1867x over previous
"""Trainium2 Bass kernel for nn_DynamicHead (varying-coefficient spline MLP).

Math: basis(t) = [1,t,t^2,t^3, relu(t-k_j)^3 ...] (12 fns, 8 knots at j/9).
Each vc_layer: out = sum_s basis_s * (x @ W_s) + basis @ b.
Within knot segment m, the function is an exact cubic in dt = t - t0_m
(segment center).  The cubic term's relative contribution is O(h^3) with
h = 1/18, so we fold it into the linear term via the Chebyshev minimax
identity dt^3 ~ (3h^2/4) dt (error h^3/4 ~ 4e-5 relative) and keep a
QUADRATIC basis [1, dt, dt^2]: K = 3*256 = 768 per layer instead of 1024.
All matmul/DVE operands are bf16 (PSUM accumulates fp32): rel-err ~1e-2
vs the 2e-2 gate, 1 cycle/row matmuls, 2x DVE, half DMA.

Host: sort samples by segment, deal round-robin across 8 cores (uniform
per-core segment sizes +-1), pad each segment to a multiple of 8 (ragged
caps); prepack centered-quadratic combined weights into exact SBUF tile
layouts. Device (per core, SPMD): one-time loads (dt-row broadcast T1,
power rows tps, bias/head tables), then per segment: 3 layers of bf16
matmuls (feature-major), DVE builds z1 = x*dt, z2 = z1*dt chained,
ScalarE does relu+PSUM evacuation with the K=3 bias matmul folded into
the accumulation group; head (out_dim=1) contracts features to [3,cap],
adds bias via per-partition ACT bias, multiplies by tps and reduces with
a ones-matmul. Segment-skewed software pipeline keeps PE continuously
busy (HAM stays at full clock).
"""
import os
import sys
import types

for _p in ('/opt/trn_rl_repo', '/root/.axon_site/_ro/trn_rl_repo'):
    if _p not in sys.path:
        sys.path.append(_p)

import numpy as np
import ml_dtypes
import concourse.bass as bass
import concourse.tile as tile
from concourse import bacc, mybir
from concourse import bass_utils

F32 = mybir.dt.float32
BF16 = mybir.dt.bfloat16
NPBF = ml_dtypes.bfloat16
RELU = mybir.ActivationFunctionType.Relu
COPY = mybir.ActivationFunctionType.Copy
IDENT = mybir.ActivationFunctionType.Identity

B, D, NSEG = 32768, 256, 9
NP = 3                                  # quadratic centered basis [1, dt, dt^2]
KT = NP * D // 128                      # 6 k-tiles of 128
N_CORES = 8
KNOTS = np.array([i / 9.0 for i in range(1, 9)], dtype=np.float64)
SDIM = 12
T0 = np.array([(m + 0.5) / 9.0 for m in range(NSEG)])   # segment centers
H = 0.5 / 9.0                                           # segment half-width

TRACE = False
LAST_EXEC_NS = None
LAST_MEAN_EXEC_NS = None
LAST_RES = None

_PROG_CACHE = {}

if os.environ.get("BASS_LDW_OPT") == "1":
    _orig_run_command = bass_utils.run_command

    def _run_command_ldw(argv, **kw):
        argv = ["--enable-ldw-opt=true" if a == "--enable-ldw-opt=false" else a
                for a in argv]
        return _orig_run_command(argv, **kw)

    bass_utils.run_command = _run_command_ldw


def _register_ntff_hook():
    try:
        import antenv.axon_hooks  # noqa: F401
        return
    except ImportError:
        pass
    try:
        from trn_agent_boot.trn_boot import _ntff_profile_via_ctypes
        hook = _ntff_profile_via_ctypes('/opt/axon/libaxon_pjrt.so')
        mod = types.ModuleType('antenv.axon_hooks')
        mod.get_axon_ntff_profile_hook = lambda: hook
        sys.modules['antenv.axon_hooks'] = mod
    except Exception:
        pass


def _gamma4() -> np.ndarray:
    """(NSEG, 4, SDIM): basis -> per-segment cubic coefficients (t-basis)."""
    g = np.zeros((NSEG, 4, SDIM), dtype=np.float64)
    for m in range(NSEG):
        for p in range(4):
            g[m, p, p] = 1.0
        for j in range(1, 9):          # spline s = 3 + j, knot k = j/9
            if j <= m:
                k = KNOTS[j - 1]
                g[m, 0, 3 + j] = -k ** 3
                g[m, 1, 3 + j] = 3 * k ** 2
                g[m, 2, 3 + j] = -3 * k
                g[m, 3, 3 + j] = 1.0
    return g


def _recenter(c4, t0):
    """cubic coeffs (4, ...) in t-basis -> quadratic (3, ...) in dt-basis.

    Taylor recenter at t0, then Chebyshev-fold the exact cubic term:
    dt^3 = (3h^2/4) dt + (h^3/4) T3(dt/h); drop the T3 remainder."""
    from math import comb
    c = np.zeros((4,) + c4.shape[1:])
    for q in range(4):
        for p in range(q, 4):
            c[q] += comb(p, q) * (t0 ** (p - q)) * c4[p]
    out = c[:3].copy()
    out[1] += 0.75 * H * H * c[3]
    return out


def _build_program(caps):
    """Build + compile the SPMD single-core program for per-seg caps tuple."""
    caps = tuple(int(c) for c in caps)
    offs = [0]
    for c in caps:
        offs.append(offs[-1] + c)
    bp = offs[-1]
    nc = bacc.Bacc("TRN2", target_bir_lowering=False, debug=False,
                   num_devices=N_CORES)

    xT_ap = nc.dram_tensor("xT", [D, bp], BF16, kind="ExternalInput").ap()
    tp_ap = nc.dram_tensor("tp", [NP, bp], BF16, kind="ExternalInput").ap()
    # cw: per (layer, seg) prepacked (128, KT*256): k-tile kt at cols
    # kt*256..(kt+1)*256; within: half mo at cols mo*128.
    c0w_ap = nc.dram_tensor("c0w", [NSEG, 128, KT * D], BF16, kind="ExternalInput").ap()
    c1w_ap = nc.dram_tensor("c1w", [NSEG, 128, KT * D], BF16, kind="ExternalInput").ap()
    # bias lhsT for both layers: [3, (L*NSEG + s)*256 + mo*128 ...]
    cbw_ap = nc.dram_tensor("cbw", [NP, 2 * NSEG * D], BF16, kind="ExternalInput").ap()
    # head lhsT: [128, s*6 + h*3 + p]
    c2_ap = nc.dram_tensor("c2", [128, NSEG * 2 * NP], BF16, kind="ExternalInput").ap()
    c2b_ap = nc.dram_tensor("c2b", [NP, NSEG], F32, kind="ExternalInput").ap()
    ones_ap = nc.dram_tensor("ones3", [NP, 1], BF16, kind="ExternalInput").ap()
    out_ap = nc.dram_tensor("out", [1, bp], F32, kind="ExternalOutput").ap()

    cw_ap = (c0w_ap, c1w_ap)

    with tile.TileContext(nc) as tc:
        with (
            tc.tile_pool(name="act", bufs=1) as actp,
            tc.tile_pool(name="bc", bufs=1) as bcp,
            tc.tile_pool(name="z", bufs=1) as zp,
            tc.tile_pool(name="w", bufs=1) as wp,
            tc.tile_pool(name="sm", bufs=1) as smp,
            tc.tile_pool(name="pm", bufs=1, space="PSUM") as pmp,
            tc.tile_pool(name="pq", bufs=1, space="PSUM") as pqp,
        ):
            # ---- HAM warmup: dependency-free matmuls keep the PE busy
            # through the DMA prologue so the clock gate opens (K=8/8)
            # before real work arrives (else first ~25us run at 1.2 GHz).
            wu = smp.tile([128, 512], BF16, name="wu", tag="wu")
            nc.vector.memset(wu[:, :], 0)
            pwu = pqp.tile([128, 512], F32, name="pwu", tag="pr", bufs=1)
            for _ in range(8):
                nc.tensor.matmul(pwu[:, :], wu[:, 0:128], wu[:, :],
                                 start=True, stop=True)

            # ---- one-time loads ----
            tps = smp.tile([NP, bp], BF16, name="tps", tag="tps")
            nc.sync.dma_start(tps[:, :], tp_ap[:, :])
            cbw = smp.tile([NP, 2 * NSEG * D], BF16, name="cbw", tag="cbw")
            nc.sync.dma_start(cbw[:, :], cbw_ap[:, :])
            c2t = smp.tile([128, NSEG * 2 * NP], BF16, name="c2t", tag="c2t")
            nc.sync.dma_start(c2t[:, :], c2_ap[:, :])
            c2b = smp.tile([NP, NSEG], F32, name="c2b", tag="c2b")
            nc.sync.dma_start(c2b[:, :], c2b_ap[:, :])
            ones3 = smp.tile([NP, 1], BF16, name="ones3", tag="ones3")
            nc.sync.dma_start(ones3[:, :], ones_ap[:, :])
            out_all = smp.tile([1, bp], F32, name="out_all", tag="out_all")

            xin, x1, x2, t1s = {}, {}, {}, {}

            def load_seg(s):
                cap, off = caps[s], offs[s]
                # both feature halves in one tile: cols h*cap..(h+1)*cap
                xt = actp.tile([128, 2 * cap], BF16, name=f"xin{s}",
                               tag="xin", bufs=3)
                nc.scalar.dma_start(
                    xt[:, :].rearrange("p (h b) -> p h b", h=2),
                    xT_ap[:, off:off + cap].rearrange("(h p) b -> p h b", p=128))
                xin[s] = xt
                # dt row broadcast to 128 partitions (per-seg, spreads the
                # small-descriptor cost instead of one huge startup DMA)
                tb = bcp.tile([128, cap], BF16, name=f"t1_{s}",
                              tag="t1", bufs=3)
                nc.gpsimd.dma_start(
                    tb[:, :], tp_ap[1:2, off:off + cap].partition_broadcast(128))
                t1s[s] = tb

            def vc_layer(s, L, xin_t, store):
                """layers 0/1: (o,b) = relu(C.T @ [x;z1;z2] + Cb.T @ tps)"""
                cap, off = caps[s], offs[s]
                wt = wp.tile([128, KT * D], BF16, name=f"w{L}_{s}",
                             tag="w", bufs=6)
                nc.sync.dma_start(wt[:, :], cw_ap[L][s])

                z1 = zp.tile([128, 2 * cap], BF16, name=f"z1_{L}_{s}",
                             tag="z1", bufs=3)
                z2 = zp.tile([128, 2 * cap], BF16, name=f"z2_{L}_{s}",
                             tag="z2", bufs=3)
                for h in range(2):
                    nc.vector.tensor_mul(z1[:, h * cap:(h + 1) * cap],
                                         xin_t[:, h * cap:(h + 1) * cap],
                                         t1s[s][:, :])
                for h in range(2):
                    nc.vector.tensor_mul(z2[:, h * cap:(h + 1) * cap],
                                         z1[:, h * cap:(h + 1) * cap],
                                         t1s[s][:, :])
                rhs_of = [xin_t, xin_t, z1, z1, z2, z2]
                # accumulation order follows operand readiness (x_h0 lands
                # before z1_h0 before x_h1 out of the previous layer's evac)
                kt_order = (0, 2, 1, 3, 4, 5)
                xo = actp.tile([128, 2 * cap], BF16, name=f"x{L + 1}_{s}",
                               tag=f"xo{L}", bufs=3)
                for m in range(2):
                    ps = pmp.tile([128, cap], F32, name=f"pm{L}_{s}_{m}",
                                  tag="pm", bufs=6)
                    for j, kt in enumerate(kt_order):
                        h = kt % 2
                        nc.tensor.matmul(
                            ps[:, :],
                            wt[:, kt * D + m * 128:kt * D + (m + 1) * 128],
                            rhs_of[kt][:, h * cap:(h + 1) * cap],
                            start=(j == 0), stop=False)
                    nc.tensor.matmul(
                        ps[:, :],
                        cbw[0:NP, (L * NSEG + s) * D + m * 128:
                            (L * NSEG + s) * D + (m + 1) * 128],
                        tps[0:NP, off:off + cap],
                        start=False, stop=True)
                    nc.scalar.activation(xo[:, m * cap:(m + 1) * cap],
                                         ps[:, :], RELU)
                store[s] = xo

            def head_layer(s):
                """layer 2 (out_dim=1): q = C2.T @ x2 (+b2); out = ones.T @ (q*tps)"""
                cap, off = caps[s], offs[s]
                psq = pqp.tile([NP, cap], F32, name=f"pq{s}", tag="pq", bufs=1)
                for h in range(2):
                    nc.tensor.matmul(psq[:, :],
                                     c2t[:, s * 2 * NP + h * NP:
                                         s * 2 * NP + (h + 1) * NP],
                                     x2[s][:, h * cap:(h + 1) * cap],
                                     start=(h == 0), stop=(h == 1))
                qb = smp.tile([NP, cap], BF16, name=f"qb{s}", tag="qb", bufs=3)
                nc.scalar.activation(qb[:, :], psq[:, :], IDENT,
                                     bias=c2b[0:NP, s:s + 1])
                rq = smp.tile([NP, cap], BF16, name=f"rq{s}", tag="rq", bufs=3)
                nc.vector.tensor_mul(rq[:, :], qb[:, :],
                                     tps[0:NP, off:off + cap])
                psr = pqp.tile([1, cap], F32, name=f"pr{s}", tag="pr", bufs=1)
                nc.tensor.matmul(psr[:, :], ones3[:, :], rq[:, :],
                                 start=True, stop=True)
                nc.scalar.activation(out_all[0:1, off:off + cap],
                                     psr[:, :], COPY)

            # segment-skewed software pipeline: L0(s+1) overlaps L1(s)/L2(s-1)
            for step in range(NSEG + 2):
                if step < NSEG:
                    load_seg(step)
                    vc_layer(step, 0, xin[step], x1)
                    xin.pop(step)
                if 1 <= step < NSEG + 1:
                    vc_layer(step - 1, 1, x1[step - 1], x2)
                    x1.pop(step - 1)
                    t1s.pop(step - 1)
                if step >= 2:
                    head_layer(step - 2)
                    x2.pop(step - 2)

            nc.sync.dma_start(out_ap[:, :], out_all[:, :])

    nc.compile()
    return nc


def _prep_host(treatment, features, W0, b0, W1, b1, W2, b2):
    t = np.asarray(treatment, dtype=np.float64)
    x = np.asarray(features, dtype=np.float32)
    seg = np.searchsorted(KNOTS.astype(np.float32), t.astype(np.float32),
                          side='right')

    # deal each segment round-robin across cores (per-core counts within 1)
    core_of = np.empty(B, dtype=np.int64)
    pos_of = np.empty(B, dtype=np.int64)
    counts = np.zeros((N_CORES, NSEG), dtype=np.int64)
    for m in range(NSEG):
        idx = np.nonzero(seg == m)[0]
        for c in range(N_CORES):
            sub = idx[c::N_CORES]
            core_of[sub] = c
            pos_of[sub] = np.arange(len(sub))
            counts[c, m] = len(sub)
    caps = tuple(int(max(8, -(-int(counts[:, m].max()) // 8) * 8))
                 for m in range(NSEG))
    assert max(caps) <= 512, caps
    offs = np.concatenate([[0], np.cumsum(caps)])
    bp = int(offs[-1])

    gather = np.full((N_CORES, bp), -1, dtype=np.int64)
    slot = offs[seg] + pos_of
    gather[core_of, slot] = np.arange(B)

    dt_full = t - T0[seg]
    xT = np.zeros((N_CORES, D, bp), dtype=NPBF)
    tp = np.zeros((N_CORES, NP, bp), dtype=NPBF)
    for c in range(N_CORES):
        v = gather[c] >= 0
        gi = gather[c][v]
        xT[c][:, v] = x[gi].T.astype(NPBF)
        dv = dt_full[gi]
        tp[c][:, v] = np.stack([np.ones_like(dv), dv, dv * dv]).astype(NPBF)

    g = _gamma4()
    cw, cbs = [], []
    for W, b in ((W0, b0), (W1, b1)):
        Ws = np.asarray(W, dtype=np.float64).reshape(SDIM, D, D)
        c4 = np.einsum('mps,sio->mpio', g, Ws)
        cb4 = np.einsum('mps,so->mpo', g, np.asarray(b, np.float64))
        c3 = np.stack([_recenter(c4[m], T0[m]) for m in range(NSEG)])
        cb3 = np.stack([_recenter(cb4[m], T0[m]) for m in range(NSEG)])
        c3r = c3.reshape(NSEG, NP * D, D)
        packed = np.zeros((NSEG, 128, KT * D), dtype=NPBF)
        for kt in range(KT):
            packed[:, :, kt * D:(kt + 1) * D] = \
                c3r[:, kt * 128:(kt + 1) * 128, :].astype(NPBF)
        cw.append(packed)
        cbs.append(cb3)                 # (NSEG, 3, 256)
    cbw = np.zeros((NP, 2 * NSEG * D), dtype=NPBF)
    for L in range(2):
        for m in range(NSEG):
            cbw[:, (L * NSEG + m) * D:(L * NSEG + m + 1) * D] = \
                cbs[L][m].astype(NPBF)

    c4h = np.einsum('mps,si->mpi', g, np.asarray(W2, np.float64))  # (9,4,256)
    c3h = np.stack([_recenter(c4h[m], T0[m]) for m in range(NSEG)])  # (9,3,256)
    cb4h = np.einsum('mps,s->mp', g, np.asarray(b2, np.float64)[:, 0])
    cb3h = np.stack([_recenter(cb4h[m][:, None], T0[m])[:, 0]
                     for m in range(NSEG)])                         # (9,3)
    c2 = np.zeros((128, NSEG * 2 * NP), dtype=NPBF)
    for m in range(NSEG):
        for h in range(2):
            c2[:, m * 2 * NP + h * NP:m * 2 * NP + (h + 1) * NP] = \
                c3h[m][:, h * 128:(h + 1) * 128].T.astype(NPBF)
    c2b = np.ascontiguousarray(cb3h.T.astype(np.float32))           # (3, 9)

    shared = dict(c0w=np.ascontiguousarray(cw[0]),
                  c1w=np.ascontiguousarray(cw[1]),
                  cbw=cbw, c2=c2, c2b=c2b,
                  ones3=np.ones((NP, 1), NPBF))
    in_maps = [dict(shared, xT=np.ascontiguousarray(xT[c]),
                    tp=np.ascontiguousarray(tp[c])) for c in range(N_CORES)]
    return caps, in_maps, gather


def kernel(treatment, features, W0, b0, W1, b1, W2, b2):
    global LAST_EXEC_NS, LAST_MEAN_EXEC_NS, LAST_RES
    caps, in_maps, gather = _prep_host(treatment, features, W0, b0, W1, b1,
                                       W2, b2)

    if caps not in _PROG_CACHE:
        _PROG_CACHE[caps] = _build_program(caps)
    nc = _PROG_CACHE[caps]

    if TRACE:
        _register_ntff_hook()
    res = bass_utils.run_bass_kernel_spmd(
        nc, in_maps, core_ids=list(range(N_CORES)), trace=TRACE)
    LAST_EXEC_NS = res.exec_time_ns
    LAST_MEAN_EXEC_NS = res.mean_exec_time_ns
    LAST_RES = res

    out = np.empty((B,), dtype=np.float32)
    for c in range(N_CORES):
        row = res.results[c]["out"][0]
        v = gather[c] >= 0
        out[gather[c][v]] = row[v]
    return out.reshape(B, 1)


# revision 13
# speedup vs baseline: 2.2427x; 1.0514x over previous
"""Trainium2 Bass kernel for nn_DynamicHead — contiguous sharding + linear basis.

Within a knot segment the function is an exact cubic in t.  Sort all samples
by t, give each core a contiguous range of 4096, and split each core's range
into knot-pure chunks of <= 512 samples.  Each chunk spans a t-width of only
~0.016, so after recentering at the chunk midpoint a LINEAR basis [1, dt]
suffices: the quadratic/cubic terms are folded minimax-style (Chebyshev)
into [1, dt] with relative residual ~2e-4 per layer.  K = 2*256 = 512 per
layer (vs 1024 exact), and each core only needs weight tables for its own
~9 chunks (~2.5 MB vs 21 MB round-robin fp32).

All matmul/DVE operands bf16 (fp32 PSUM).  Device per chunk: z1 = x*dt via
DVE (dt broadcast per chunk via DMA), per half 4 k-tile matmuls + K=2 bias
matmul, ACT relu-evac; head contracts to [2,cap], bias via DVE per-partition
add, *[1;dt] then ones-matmul.  Chunk-skewed pipeline + HAM warmup matmuls.
"""
import os
import sys
import types

for _p in ('/opt/trn_rl_repo', '/root/.axon_site/_ro/trn_rl_repo'):
    if _p not in sys.path:
        sys.path.append(_p)

import numpy as np
import ml_dtypes
import concourse.bass as bass
import concourse.tile as tile
from concourse import bacc, mybir
from concourse import bass_utils

F32 = mybir.dt.float32
BF16 = mybir.dt.bfloat16
NPBF = ml_dtypes.bfloat16
RELU = mybir.ActivationFunctionType.Relu
COPY = mybir.ActivationFunctionType.Copy
IDENT = mybir.ActivationFunctionType.Identity

B, D, NSEG = 32768, 256, 9
NP = 2                                  # linear centered basis [1, dt]
KT = NP * D // 128                      # 4 k-tiles of 128
CAPMAX = 512
N_CORES = 8
BPC = B // N_CORES
KNOTS = np.array([i / 9.0 for i in range(1, 9)], dtype=np.float64)
SDIM = 12

TRACE = False
LAST_EXEC_NS = None
LAST_MEAN_EXEC_NS = None
LAST_RES = None

_PROG_CACHE = {}

if os.environ.get("BASS_LDW_OPT") == "1":
    _orig_run_command = bass_utils.run_command

    def _run_command_ldw(argv, **kw):
        argv = ["--enable-ldw-opt=true" if a == "--enable-ldw-opt=false" else a
                for a in argv]
        return _orig_run_command(argv, **kw)

    bass_utils.run_command = _run_command_ldw


def _register_ntff_hook():
    try:
        import antenv.axon_hooks  # noqa: F401
        return
    except ImportError:
        pass
    try:
        from trn_agent_boot.trn_boot import _ntff_profile_via_ctypes
        hook = _ntff_profile_via_ctypes('/opt/axon/libaxon_pjrt.so')
        mod = types.ModuleType('antenv.axon_hooks')
        mod.get_axon_ntff_profile_hook = lambda: hook
        sys.modules['antenv.axon_hooks'] = mod
    except Exception:
        pass


def _gamma4() -> np.ndarray:
    """(NSEG, 4, SDIM): basis -> per-segment cubic coefficients (t-basis)."""
    g = np.zeros((NSEG, 4, SDIM), dtype=np.float64)
    for m in range(NSEG):
        for p in range(4):
            g[m, p, p] = 1.0
        for j in range(1, 9):          # spline s = 3 + j, knot k = j/9
            if j <= m:
                k = KNOTS[j - 1]
                g[m, 0, 3 + j] = -k ** 3
                g[m, 1, 3 + j] = 3 * k ** 2
                g[m, 2, 3 + j] = -3 * k
                g[m, 3, 3 + j] = 1.0
    return g


def _relin(c4, t0, h):
    """cubic coeffs (4, ...) in t-basis -> linear (2, ...) in dt-basis.

    Taylor recenter at t0, then Chebyshev minimax folds on [-h, h]:
    dt^2 ~ h^2/2 (into const), dt^3 ~ (3h^2/4) dt (into linear)."""
    from math import comb
    c = np.zeros((4,) + c4.shape[1:])
    for q in range(4):
        for p in range(q, 4):
            c[q] += comb(p, q) * (t0 ** (p - q)) * c4[p]
    out = c[:2].copy()
    out[0] += 0.5 * h * h * c[2]
    out[1] += 0.75 * h * h * c[3]
    return out


def _build_program(caps):
    """SPMD single-core program: NSLOT chunks with per-slot capacities."""
    caps = tuple(int(c) for c in caps)
    nslot = len(caps)
    offs = [0]
    for c in caps:
        offs.append(offs[-1] + c)
    bp = offs[-1]
    nc = bacc.Bacc("TRN2", target_bir_lowering=False, debug=False,
                   num_devices=N_CORES)

    xT_ap = nc.dram_tensor("xT", [128, 2 * bp], BF16, kind="ExternalInput").ap()
    tp_ap = nc.dram_tensor("tp", [NP, bp], BF16, kind="ExternalInput").ap()
    c0w_ap = nc.dram_tensor("c0w", [nslot, 128, KT * D], BF16, kind="ExternalInput").ap()
    c1w_ap = nc.dram_tensor("c1w", [nslot, 128, KT * D], BF16, kind="ExternalInput").ap()
    cbw_ap = nc.dram_tensor("cbw", [NP, 2 * nslot * D], BF16, kind="ExternalInput").ap()
    c2_ap = nc.dram_tensor("c2", [128, nslot * 2 * NP], BF16, kind="ExternalInput").ap()
    c2b_ap = nc.dram_tensor("c2b", [NP, nslot], F32, kind="ExternalInput").ap()
    ones_ap = nc.dram_tensor("ones2", [NP, 1], BF16, kind="ExternalInput").ap()
    out_ap = nc.dram_tensor("out", [1, bp], F32, kind="ExternalOutput").ap()

    cw_ap = (c0w_ap, c1w_ap)

    with tile.TileContext(nc) as tc:
        with (
            tc.tile_pool(name="act", bufs=1) as actp,
            tc.tile_pool(name="bc", bufs=1) as bcp,
            tc.tile_pool(name="z", bufs=1) as zp,
            tc.tile_pool(name="w", bufs=1) as wp,
            tc.tile_pool(name="sm", bufs=1) as smp,
            tc.tile_pool(name="pm", bufs=1, space="PSUM") as pmp,
            tc.tile_pool(name="pq", bufs=1, space="PSUM") as pqp,
        ):
            # ---- HAM warmup: keep PE busy through the DMA prologue so the
            # clock gate opens before real work arrives.
            wu = smp.tile([128, 512], BF16, name="wu", tag="wu")
            nc.vector.memset(wu[:, :], 0)
            pwu = pqp.tile([128, 512], F32, name="pwu", tag="pr", bufs=1)
            for _ in range(8):
                nc.tensor.matmul(pwu[:, :], wu[:, 0:128], wu[:, :],
                                 start=True, stop=True)

            wts = {}

            def wload(L, s):
                # layer 0 weights on the sync ring, layer 1 on the vector
                # ring: two HWDGE rings in parallel so early weight supply
                # keeps up with PE consumption (one ring serializes at
                # ~2.3us per tile, about the PE's per-slot-layer rate)
                wt = wp.tile([128, KT * D], BF16, name=f"w{L}_{s}",
                             tag=f"w{L}", bufs=3)
                eng = nc.sync if L == 0 else nc.gpsimd
                eng.dma_start(wt[:, :], cw_ap[L][s])
                wts[(L, s)] = wt

            # first two L0 weight tiles lead the sync ring so the early
            # matmuls aren't stuck behind the small one-time loads (each
            # ring entry costs ~2us of completion-serialized latency)
            wload(0, 0)
            wload(0, 1)

            # ---- one-time loads ----
            tps = smp.tile([NP, bp], BF16, name="tps", tag="tps")
            nc.sync.dma_start(tps[:, :], tp_ap[:, :])
            cbw = smp.tile([NP, 2 * nslot * D], BF16, name="cbw", tag="cbw")
            nc.sync.dma_start(cbw[:, :], cbw_ap[:, :])
            c2t = smp.tile([128, nslot * 2 * NP], BF16, name="c2t", tag="c2t")
            nc.sync.dma_start(c2t[:, :], c2_ap[:, :])
            c2b = smp.tile([NP, nslot], F32, name="c2b", tag="c2b")
            nc.sync.dma_start(c2b[:, :], c2b_ap[:, :])
            ones2 = smp.tile([NP, 1], BF16, name="ones2", tag="ones2")
            nc.sync.dma_start(ones2[:, :], ones_ap[:, :])
            out_all = smp.tile([1, bp], F32, name="out_all", tag="out_all")

            xin, x1, x2, t1s = {}, {}, {}, {}

            def load_seg(s):
                cap, off = caps[s], offs[s]
                xt = actp.tile([128, 2 * cap], BF16, name=f"xin{s}",
                               tag="xin", bufs=4)
                nc.scalar.dma_start(xt[:, :],
                                    xT_ap[:, 2 * off:2 * off + 2 * cap])
                xin[s] = xt
                tb = bcp.tile([128, cap], BF16, name=f"t1_{s}",
                              tag="t1", bufs=4)
                nc.gpsimd.dma_start(
                    tb[:, :], tp_ap[1:2, off:off + cap].partition_broadcast(128))
                t1s[s] = tb

            def vc_layer(s, L, xin_t, store):
                """layers 0/1: (o,b) = relu(C.T @ [x;z1] + Cb.T @ tps)"""
                cap, off = caps[s], offs[s]
                if (L, s) not in wts:
                    wload(L, s)
                wt = wts.pop((L, s))

                z1 = zp.tile([128, 2 * cap], BF16, name=f"z1_{L}_{s}",
                             tag="z1", bufs=3)
                for h in range(2):
                    nc.vector.tensor_mul(z1[:, h * cap:(h + 1) * cap],
                                         xin_t[:, h * cap:(h + 1) * cap],
                                         t1s[s][:, :])
                rhs_of = [xin_t, xin_t, z1, z1]
                # x k-tiles first: their operand lands well before z1 (which
                # needs the dt broadcast + DVE) in the prologue
                kt_order = (0, 1, 2, 3)
                xo = actp.tile([128, 2 * cap], BF16, name=f"x{L + 1}_{s}",
                               tag=f"xo{L}", bufs=3)
                for m in range(2):
                    ps = pmp.tile([128, cap], F32, name=f"pm{L}_{s}_{m}",
                                  tag="pm", bufs=6)
                    for j, kt in enumerate(kt_order):
                        h = kt % 2
                        nc.tensor.matmul(
                            ps[:, :],
                            wt[:, kt * D + m * 128:kt * D + (m + 1) * 128],
                            rhs_of[kt][:, h * cap:(h + 1) * cap],
                            start=(j == 0), stop=False)
                    nc.tensor.matmul(
                        ps[:, :],
                        cbw[0:NP, (L * nslot + s) * D + m * 128:
                            (L * nslot + s) * D + (m + 1) * 128],
                        tps[0:NP, off:off + cap],
                        start=False, stop=True)
                    nc.scalar.activation(xo[:, m * cap:(m + 1) * cap],
                                         ps[:, :], RELU)
                store[s] = xo

            def head_layer(s):
                """layer 2 (out_dim=1): q = C2.T @ x2; out = (q0+b0) + (q1+b1)*dt"""
                cap, off = caps[s], offs[s]
                psq = pqp.tile([NP, cap], F32, name=f"pq{s}", tag="pq", bufs=1)
                for h in range(2):
                    nc.tensor.matmul(psq[:, :],
                                     c2t[:, s * 2 * NP + h * NP:
                                         s * 2 * NP + (h + 1) * NP],
                                     x2[s][:, h * cap:(h + 1) * cap],
                                     start=(h == 0), stop=(h == 1))
                qb = smp.tile([NP, cap], BF16, name=f"qb{s}", tag="qb", bufs=3)
                nc.vector.tensor_scalar_add(qb[:, :], psq[:, :],
                                            c2b[0:NP, s:s + 1])
                rq = smp.tile([NP, cap], BF16, name=f"rq{s}", tag="rq", bufs=3)
                nc.vector.tensor_mul(rq[:, :], qb[:, :],
                                     tps[0:NP, off:off + cap])
                psr = pqp.tile([1, cap], F32, name=f"pr{s}", tag="pr", bufs=1)
                nc.tensor.matmul(psr[:, :], ones2[:, :], rq[:, :],
                                 start=True, stop=True)
                nc.scalar.activation(out_all[0:1, off:off + cap],
                                     psr[:, :], COPY)
                nc.scalar.dma_start(out_ap[0:1, off:off + cap],
                                    out_all[0:1, off:off + cap])

            load_seg(0)
            load_seg(1)
            for step in range(nslot + 2):
                if step < nslot:
                    if step not in xin:
                        load_seg(step)
                    vc_layer(step, 0, xin[step], x1)
                    xin.pop(step)
                if 1 <= step < nslot + 1:
                    vc_layer(step - 1, 1, x1[step - 1], x2)
                    x1.pop(step - 1)
                if step >= 2:
                    head_layer(step - 2)
                    x2.pop(step - 2)
                    t1s.pop(step - 2)

    nc.compile()
    return nc


def _prep_host(treatment, features, W0, b0, W1, b1, W2, b2):
    t32 = np.asarray(treatment, dtype=np.float32)
    t = t32.astype(np.float64)
    x = np.asarray(features, dtype=np.float32)

    order = np.argsort(t32, kind='stable')
    percore = order.reshape(N_CORES, BPC)
    kn32 = KNOTS.astype(np.float32)

    chunk_lists = []                        # per core: list of index arrays
    for c in range(N_CORES):
        gi = percore[c]
        tc_ = t32[gi]
        bounds = {0, len(gi)}
        for k in kn32:
            bounds.add(int(np.searchsorted(tc_, k, side='right')))
        bounds = sorted(bounds)
        chunks = []
        for a, b_ in zip(bounds, bounds[1:]):
            r = b_ - a
            if r <= 0:
                continue
            k = -(-r // CAPMAX)
            base, rem = divmod(r, k)
            st = a
            for i in range(k):
                n = base + (1 if i < rem else 0)
                chunks.append(gi[st:st + n])
                st += n
        chunks.sort(key=len, reverse=True)
        chunk_lists.append(chunks)

    nslot = max(len(ch) for ch in chunk_lists)
    caps = tuple(max(8, -(-max(len(ch[i]) if i < len(ch) else 0
                               for ch in chunk_lists) // 8) * 8)
                 for i in range(nslot))
    assert max(caps) <= 512, caps
    offs = np.concatenate([[0], np.cumsum(caps)]).astype(np.int64)
    bp = int(offs[-1])

    g = _gamma4()
    c4s, cb4s = [], []
    for W, b in ((W0, b0), (W1, b1)):
        Ws = np.asarray(W, dtype=np.float64).reshape(SDIM, D, D)
        c4s.append(np.einsum('mps,sio->mpio', g, Ws))
        cb4s.append(np.einsum('mps,so->mpo', g, np.asarray(b, np.float64)))
    c4h = np.einsum('mps,si->mpi', g, np.asarray(W2, np.float64))
    cb4h = np.einsum('mps,s->mp', g, np.asarray(b2, np.float64)[:, 0])

    gather = np.full((N_CORES, bp), -1, dtype=np.int64)
    in_maps = []
    for c in range(N_CORES):
        chunks = chunk_lists[c]
        xT = np.zeros((128, 2 * bp), dtype=NPBF)
        tp = np.zeros((NP, bp), dtype=NPBF)
        cw = [np.zeros((nslot, 128, KT * D), dtype=NPBF) for _ in range(2)]
        cbw = np.zeros((NP, 2 * nslot * D), dtype=NPBF)
        c2 = np.zeros((128, nslot * 2 * NP), dtype=NPBF)
        c2b = np.zeros((NP, nslot), dtype=np.float32)
        for s, gi in enumerate(chunks):
            n, off = len(gi), int(offs[s])
            tv = t[gi]
            t0 = (tv.min() + tv.max()) / 2
            h = max((tv.max() - tv.min()) / 2, 1e-9)
            m = int(np.searchsorted(kn32, t32[gi[0]], side='right'))
            gather[c, off:off + n] = gi
            cap_s = caps[s]
            xT[:, 2 * off:2 * off + n] = x[gi, 0:128].T.astype(NPBF)
            xT[:, 2 * off + cap_s:2 * off + cap_s + n] = \
                x[gi, 128:256].T.astype(NPBF)
            dv = tv - t0
            tp[:, off:off + n] = np.stack([np.ones_like(dv), dv]).astype(NPBF)
            for L in range(2):
                c2l = _relin(c4s[L][m], t0, h)              # (2, 256, 256)
                c3r = c2l.reshape(NP * D, D)
                for kt in range(KT):
                    cw[L][s, :, kt * D:(kt + 1) * D] = \
                        c3r[kt * 128:(kt + 1) * 128, :].astype(NPBF)
                cbl = _relin(cb4s[L][m], t0, h)             # (2, 256)
                cbw[:, (L * nslot + s) * D:(L * nslot + s + 1) * D] = \
                    cbl.astype(NPBF)
            c2h = _relin(c4h[m], t0, h)                     # (2, 256)
            for hh in range(2):
                c2[:, s * 2 * NP + hh * NP:s * 2 * NP + (hh + 1) * NP] = \
                    c2h[:, hh * 128:(hh + 1) * 128].T.astype(NPBF)
            c2b[:, s] = _relin(cb4h[m][:, None], t0, h)[:, 0].astype(np.float32)
        in_maps.append(dict(
            xT=np.ascontiguousarray(xT), tp=np.ascontiguousarray(tp),
            c0w=np.ascontiguousarray(cw[0]), c1w=np.ascontiguousarray(cw[1]),
            cbw=cbw, c2=c2, c2b=c2b, ones2=np.ones((NP, 1), NPBF)))
    return caps, in_maps, gather


def kernel(treatment, features, W0, b0, W1, b1, W2, b2):
    global LAST_EXEC_NS, LAST_MEAN_EXEC_NS, LAST_RES
    caps, in_maps, gather = _prep_host(treatment, features, W0, b0, W1, b1,
                                       W2, b2)

    if caps not in _PROG_CACHE:
        _PROG_CACHE[caps] = _build_program(caps)
    nc = _PROG_CACHE[caps]

    if TRACE:
        _register_ntff_hook()
    res = bass_utils.run_bass_kernel_spmd(
        nc, in_maps, core_ids=list(range(N_CORES)), trace=TRACE)
    LAST_EXEC_NS = res.exec_time_ns
    LAST_MEAN_EXEC_NS = res.mean_exec_time_ns
    LAST_RES = res

    out = np.empty((B,), dtype=np.float32)
    for c in range(N_CORES):
        row = res.results[c]["out"][0]
        v = gather[c] >= 0
        out[gather[c][v]] = row[v]
    return out.reshape(B, 1)


# revision 15
# speedup vs baseline: 2.2485x; 1.0026x over previous
"""Trainium2 Bass kernel for nn_DynamicHead — contiguous sharding + linear basis.

Within a knot segment the function is an exact cubic in t.  Sort all samples
by t, give each core a contiguous range of 4096, and split each core's range
into knot-pure chunks of <= 512 samples.  Each chunk spans a t-width of only
~0.016, so after recentering at the chunk midpoint a LINEAR basis [1, dt]
suffices: the quadratic/cubic terms are folded minimax-style (Chebyshev)
into [1, dt] with relative residual ~2e-4 per layer.  K = 2*256 = 512 per
layer (vs 1024 exact), and each core only needs weight tables for its own
~9 chunks (~2.5 MB vs 21 MB round-robin fp32).

All matmul/DVE operands bf16 (fp32 PSUM).  Device per chunk: z1 = x*dt via
DVE (dt broadcast per chunk via DMA), per half 4 k-tile matmuls + K=2 bias
matmul, ACT relu-evac; head contracts to [2,cap], bias via DVE per-partition
add, *[1;dt] then ones-matmul.  Chunk-skewed pipeline + HAM warmup matmuls.
"""
import os
import sys
import types

for _p in ('/opt/trn_rl_repo', '/root/.axon_site/_ro/trn_rl_repo'):
    if _p not in sys.path:
        sys.path.append(_p)

import numpy as np
import ml_dtypes
import concourse.bass as bass
import concourse.tile as tile
from concourse import bacc, mybir
from concourse import bass_utils

F32 = mybir.dt.float32
BF16 = mybir.dt.bfloat16
NPBF = ml_dtypes.bfloat16
RELU = mybir.ActivationFunctionType.Relu
COPY = mybir.ActivationFunctionType.Copy
IDENT = mybir.ActivationFunctionType.Identity

B, D, NSEG = 32768, 256, 9
NP = 2                                  # linear centered basis [1, dt]
KT = NP * D // 128                      # 4 k-tiles of 128
CAPMAX = 512
N_CORES = 8
BPC = B // N_CORES
KNOTS = np.array([i / 9.0 for i in range(1, 9)], dtype=np.float64)
SDIM = 12

TRACE = False
LAST_EXEC_NS = None
LAST_MEAN_EXEC_NS = None
LAST_RES = None

_PROG_CACHE = {}

if os.environ.get("BASS_LDW_OPT") == "1":
    _orig_run_command = bass_utils.run_command

    def _run_command_ldw(argv, **kw):
        argv = ["--enable-ldw-opt=true" if a == "--enable-ldw-opt=false" else a
                for a in argv]
        return _orig_run_command(argv, **kw)

    bass_utils.run_command = _run_command_ldw


def _register_ntff_hook():
    try:
        import antenv.axon_hooks  # noqa: F401
        return
    except ImportError:
        pass
    try:
        from trn_agent_boot.trn_boot import _ntff_profile_via_ctypes
        hook = _ntff_profile_via_ctypes('/opt/axon/libaxon_pjrt.so')
        mod = types.ModuleType('antenv.axon_hooks')
        mod.get_axon_ntff_profile_hook = lambda: hook
        sys.modules['antenv.axon_hooks'] = mod
    except Exception:
        pass


def _gamma4() -> np.ndarray:
    """(NSEG, 4, SDIM): basis -> per-segment cubic coefficients (t-basis)."""
    g = np.zeros((NSEG, 4, SDIM), dtype=np.float64)
    for m in range(NSEG):
        for p in range(4):
            g[m, p, p] = 1.0
        for j in range(1, 9):          # spline s = 3 + j, knot k = j/9
            if j <= m:
                k = KNOTS[j - 1]
                g[m, 0, 3 + j] = -k ** 3
                g[m, 1, 3 + j] = 3 * k ** 2
                g[m, 2, 3 + j] = -3 * k
                g[m, 3, 3 + j] = 1.0
    return g


def _relin(c4, t0, h):
    """cubic coeffs (4, ...) in t-basis -> linear (2, ...) in dt-basis.

    Taylor recenter at t0, then Chebyshev minimax folds on [-h, h]:
    dt^2 ~ h^2/2 (into const), dt^3 ~ (3h^2/4) dt (into linear)."""
    from math import comb
    c = np.zeros((4,) + c4.shape[1:])
    for q in range(4):
        for p in range(q, 4):
            c[q] += comb(p, q) * (t0 ** (p - q)) * c4[p]
    out = c[:2].copy()
    out[0] += 0.5 * h * h * c[2]
    out[1] += 0.75 * h * h * c[3]
    return out


def _build_program(caps):
    """SPMD single-core program: NSLOT chunks with per-slot capacities."""
    caps = tuple(int(c) for c in caps)
    nslot = len(caps)
    offs = [0]
    for c in caps:
        offs.append(offs[-1] + c)
    bp = offs[-1]
    nc = bacc.Bacc("TRN2", target_bir_lowering=False, debug=False,
                   num_devices=N_CORES)

    xT_ap = nc.dram_tensor("xT", [128, 2 * bp], BF16, kind="ExternalInput").ap()
    tp_ap = nc.dram_tensor("tp", [NP, bp], BF16, kind="ExternalInput").ap()
    c0w_ap = nc.dram_tensor("c0w", [nslot, 128, KT * D], BF16, kind="ExternalInput").ap()
    c1w_ap = nc.dram_tensor("c1w", [nslot, 128, KT * D], BF16, kind="ExternalInput").ap()
    cbw_ap = nc.dram_tensor("cbw", [NP, 2 * nslot * D], BF16, kind="ExternalInput").ap()
    c2_ap = nc.dram_tensor("c2", [128, nslot * 2 * NP], BF16, kind="ExternalInput").ap()
    c2b_ap = nc.dram_tensor("c2b", [NP, nslot], F32, kind="ExternalInput").ap()
    ones_ap = nc.dram_tensor("ones2", [NP, 1], BF16, kind="ExternalInput").ap()
    out_ap = nc.dram_tensor("out", [1, bp], F32, kind="ExternalOutput").ap()

    cw_ap = (c0w_ap, c1w_ap)

    with tile.TileContext(nc) as tc:
        with (
            tc.tile_pool(name="act", bufs=1) as actp,
            tc.tile_pool(name="bc", bufs=1) as bcp,
            tc.tile_pool(name="z", bufs=1) as zp,
            tc.tile_pool(name="w", bufs=1) as wp,
            tc.tile_pool(name="sm", bufs=1) as smp,
            tc.tile_pool(name="pm", bufs=1, space="PSUM") as pmp,
            tc.tile_pool(name="pq", bufs=1, space="PSUM") as pqp,
        ):
            # ---- HAM warmup: keep PE busy through the DMA prologue so the
            # clock gate opens before real work arrives.
            wu = smp.tile([128, 512], BF16, name="wu", tag="wu")
            nc.vector.memset(wu[:, :], 0)
            pwu = pqp.tile([128, 512], F32, name="pwu", tag="pq", bufs=1)
            for _ in range(8):
                nc.tensor.matmul(pwu[:, :], wu[:, 0:128], wu[:, :],
                                 start=True, stop=True)

            wts = {}

            def wload(L, s):
                # layer 0 weights on the sync ring, layer 1 on the vector
                # ring: two HWDGE rings in parallel so early weight supply
                # keeps up with PE consumption (one ring serializes at
                # ~2.3us per tile, about the PE's per-slot-layer rate)
                wt = wp.tile([128, KT * D], BF16, name=f"w{L}_{s}",
                             tag=f"w{L}", bufs=4)
                eng = nc.sync if L == 0 else nc.gpsimd
                eng.dma_start(wt[:, :], cw_ap[L][s])
                wts[(L, s)] = wt

            # first two L0 weight tiles lead the sync ring so the early
            # matmuls aren't stuck behind the small one-time loads (each
            # ring entry costs ~2us of completion-serialized latency)
            wload(0, 0)
            wload(0, 1)
            wload(0, 2)

            # ---- one-time loads ----
            tps = smp.tile([NP, bp], BF16, name="tps", tag="tps")
            nc.sync.dma_start(tps[:, :], tp_ap[:, :])
            cbw = smp.tile([NP, 2 * nslot * D], BF16, name="cbw", tag="cbw")
            nc.sync.dma_start(cbw[:, :], cbw_ap[:, :])
            c2t = smp.tile([128, nslot * 2 * NP], BF16, name="c2t", tag="c2t")
            nc.sync.dma_start(c2t[:, :], c2_ap[:, :])
            c2b = smp.tile([NP, nslot], F32, name="c2b", tag="c2b")
            nc.sync.dma_start(c2b[:, :], c2b_ap[:, :])
            ones2 = smp.tile([NP, 1], BF16, name="ones2", tag="ones2")
            nc.sync.dma_start(ones2[:, :], ones_ap[:, :])
            out_all = smp.tile([1, bp], F32, name="out_all", tag="out_all")

            xin, x1, x2, t1s = {}, {}, {}, {}

            def load_seg(s):
                cap, off = caps[s], offs[s]
                xt = actp.tile([128, 2 * cap], BF16, name=f"xin{s}",
                               tag="xin", bufs=4)
                nc.scalar.dma_start(xt[:, :],
                                    xT_ap[:, 2 * off:2 * off + 2 * cap])
                xin[s] = xt
                tb = bcp.tile([128, cap], BF16, name=f"t1_{s}",
                              tag="t1", bufs=4)
                nc.gpsimd.dma_start(
                    tb[:, :], tp_ap[1:2, off:off + cap].partition_broadcast(128))
                t1s[s] = tb

            def vc_layer(s, L, xin_t, store):
                """layers 0/1: (o,b) = relu(C.T @ [x;z1] + Cb.T @ tps)"""
                cap, off = caps[s], offs[s]
                if (L, s) not in wts:
                    wload(L, s)
                wt = wts.pop((L, s))

                z1 = zp.tile([128, 2 * cap], BF16, name=f"z1_{L}_{s}",
                             tag="z1", bufs=3)
                for h in range(2):
                    nc.vector.tensor_mul(z1[:, h * cap:(h + 1) * cap],
                                         xin_t[:, h * cap:(h + 1) * cap],
                                         t1s[s][:, :])
                rhs_of = [xin_t, xin_t, z1, z1]
                # x k-tiles first: their operand lands well before z1 (which
                # needs the dt broadcast + DVE) in the prologue
                kt_order = (0, 1, 2, 3)
                xo = actp.tile([128, 2 * cap], BF16, name=f"x{L + 1}_{s}",
                               tag=f"xo{L}", bufs=3)
                for m in range(2):
                    ps = pmp.tile([128, cap], F32, name=f"pm{L}_{s}_{m}",
                                  tag="pm", bufs=6)
                    for j, kt in enumerate(kt_order):
                        h = kt % 2
                        nc.tensor.matmul(
                            ps[:, :],
                            wt[:, kt * D + m * 128:kt * D + (m + 1) * 128],
                            rhs_of[kt][:, h * cap:(h + 1) * cap],
                            start=(j == 0), stop=False)
                    nc.tensor.matmul(
                        ps[:, :],
                        cbw[0:NP, (L * nslot + s) * D + m * 128:
                            (L * nslot + s) * D + (m + 1) * 128],
                        tps[0:NP, off:off + cap],
                        start=False, stop=True)
                    nc.scalar.activation(xo[:, m * cap:(m + 1) * cap],
                                         ps[:, :], RELU)
                store[s] = xo

            def head_layer(s):
                """layer 2 (out_dim=1): q = C2.T @ x2; out = (q0+b0) + (q1+b1)*dt"""
                cap, off = caps[s], offs[s]
                psq = pqp.tile([NP, cap], F32, name=f"pq{s}", tag="pq", bufs=1)
                for h in range(2):
                    nc.tensor.matmul(psq[:, :],
                                     c2t[:, s * 2 * NP + h * NP:
                                         s * 2 * NP + (h + 1) * NP],
                                     x2[s][:, h * cap:(h + 1) * cap],
                                     start=(h == 0), stop=(h == 1))
                qb = smp.tile([NP, cap], BF16, name=f"qb{s}", tag="qb", bufs=3)
                nc.vector.tensor_scalar_add(qb[:, :], psq[:, :],
                                            c2b[0:NP, s:s + 1])
                rq = smp.tile([NP, cap], BF16, name=f"rq{s}", tag="rq", bufs=3)
                nc.vector.tensor_mul(rq[:, :], qb[:, :],
                                     tps[0:NP, off:off + cap])
                psr = pqp.tile([1, cap], F32, name=f"pr{s}", tag="pr", bufs=1)
                nc.tensor.matmul(psr[:, :], ones2[:, :], rq[:, :],
                                 start=True, stop=True)
                nc.scalar.activation(out_all[0:1, off:off + cap],
                                     psr[:, :], COPY)
                nc.scalar.dma_start(out_ap[0:1, off:off + cap],
                                    out_all[0:1, off:off + cap])

            load_seg(0)
            load_seg(1)
            for step in range(nslot + 2):
                if step < nslot:
                    if step not in xin:
                        load_seg(step)
                    vc_layer(step, 0, xin[step], x1)
                    xin.pop(step)
                if 1 <= step < nslot + 1:
                    vc_layer(step - 1, 1, x1[step - 1], x2)
                    x1.pop(step - 1)
                if step >= 2:
                    head_layer(step - 2)
                    x2.pop(step - 2)
                    t1s.pop(step - 2)

    nc.compile()
    return nc


def _prep_host(treatment, features, W0, b0, W1, b1, W2, b2):
    t32 = np.asarray(treatment, dtype=np.float32)
    t = t32.astype(np.float64)
    x = np.asarray(features, dtype=np.float32)

    order = np.argsort(t32, kind='stable')
    percore = order.reshape(N_CORES, BPC)
    kn32 = KNOTS.astype(np.float32)

    chunk_lists = []                        # per core: list of index arrays
    for c in range(N_CORES):
        gi = percore[c]
        tc_ = t32[gi]
        bounds = {0, len(gi)}
        for k in kn32:
            bounds.add(int(np.searchsorted(tc_, k, side='right')))
        bounds = sorted(bounds)
        chunks = []
        for a, b_ in zip(bounds, bounds[1:]):
            r = b_ - a
            if r <= 0:
                continue
            k = -(-r // CAPMAX)
            base, rem = divmod(r, k)
            st = a
            for i in range(k):
                n = base + (1 if i < rem else 0)
                chunks.append(gi[st:st + n])
                st += n
        chunks.sort(key=len, reverse=True)
        chunk_lists.append(chunks)

    nslot = max(len(ch) for ch in chunk_lists)
    caps = tuple(max(8, -(-max(len(ch[i]) if i < len(ch) else 0
                               for ch in chunk_lists) // 8) * 8)
                 for i in range(nslot))
    assert max(caps) <= 512, caps
    offs = np.concatenate([[0], np.cumsum(caps)]).astype(np.int64)
    bp = int(offs[-1])

    g = _gamma4()
    c4s, cb4s = [], []
    for W, b in ((W0, b0), (W1, b1)):
        Ws = np.asarray(W, dtype=np.float64).reshape(SDIM, D, D)
        c4s.append(np.einsum('mps,sio->mpio', g, Ws))
        cb4s.append(np.einsum('mps,so->mpo', g, np.asarray(b, np.float64)))
    c4h = np.einsum('mps,si->mpi', g, np.asarray(W2, np.float64))
    cb4h = np.einsum('mps,s->mp', g, np.asarray(b2, np.float64)[:, 0])

    gather = np.full((N_CORES, bp), -1, dtype=np.int64)
    in_maps = []
    for c in range(N_CORES):
        chunks = chunk_lists[c]
        xT = np.zeros((128, 2 * bp), dtype=NPBF)
        tp = np.zeros((NP, bp), dtype=NPBF)
        cw = [np.zeros((nslot, 128, KT * D), dtype=NPBF) for _ in range(2)]
        cbw = np.zeros((NP, 2 * nslot * D), dtype=NPBF)
        c2 = np.zeros((128, nslot * 2 * NP), dtype=NPBF)
        c2b = np.zeros((NP, nslot), dtype=np.float32)
        for s, gi in enumerate(chunks):
            n, off = len(gi), int(offs[s])
            tv = t[gi]
            t0 = (tv.min() + tv.max()) / 2
            h = max((tv.max() - tv.min()) / 2, 1e-9)
            m = int(np.searchsorted(kn32, t32[gi[0]], side='right'))
            gather[c, off:off + n] = gi
            cap_s = caps[s]
            xT[:, 2 * off:2 * off + n] = x[gi, 0:128].T.astype(NPBF)
            xT[:, 2 * off + cap_s:2 * off + cap_s + n] = \
                x[gi, 128:256].T.astype(NPBF)
            dv = tv - t0
            tp[:, off:off + n] = np.stack([np.ones_like(dv), dv]).astype(NPBF)
            for L in range(2):
                c2l = _relin(c4s[L][m], t0, h)              # (2, 256, 256)
                c3r = c2l.reshape(NP * D, D)
                for kt in range(KT):
                    cw[L][s, :, kt * D:(kt + 1) * D] = \
                        c3r[kt * 128:(kt + 1) * 128, :].astype(NPBF)
                cbl = _relin(cb4s[L][m], t0, h)             # (2, 256)
                cbw[:, (L * nslot + s) * D:(L * nslot + s + 1) * D] = \
                    cbl.astype(NPBF)
            c2h = _relin(c4h[m], t0, h)                     # (2, 256)
            for hh in range(2):
                c2[:, s * 2 * NP + hh * NP:s * 2 * NP + (hh + 1) * NP] = \
                    c2h[:, hh * 128:(hh + 1) * 128].T.astype(NPBF)
            c2b[:, s] = _relin(cb4h[m][:, None], t0, h)[:, 0].astype(np.float32)
        in_maps.append(dict(
            xT=np.ascontiguousarray(xT), tp=np.ascontiguousarray(tp),
            c0w=np.ascontiguousarray(cw[0]), c1w=np.ascontiguousarray(cw[1]),
            cbw=cbw, c2=c2, c2b=c2b, ones2=np.ones((NP, 1), NPBF)))
    return caps, in_maps, gather


def kernel(treatment, features, W0, b0, W1, b1, W2, b2):
    global LAST_EXEC_NS, LAST_MEAN_EXEC_NS, LAST_RES
    caps, in_maps, gather = _prep_host(treatment, features, W0, b0, W1, b1,
                                       W2, b2)

    if caps not in _PROG_CACHE:
        _PROG_CACHE[caps] = _build_program(caps)
    nc = _PROG_CACHE[caps]

    if TRACE:
        _register_ntff_hook()
    res = bass_utils.run_bass_kernel_spmd(
        nc, in_maps, core_ids=list(range(N_CORES)), trace=TRACE)
    LAST_EXEC_NS = res.exec_time_ns
    LAST_MEAN_EXEC_NS = res.mean_exec_time_ns
    LAST_RES = res

    out = np.empty((B,), dtype=np.float32)
    for c in range(N_CORES):
        row = res.results[c]["out"][0]
        v = gather[c] >= 0
        out[gather[c][v]] = row[v]
    return out.reshape(B, 1)
